# revision 53
# baseline (speedup 1.0000x reference)
"""Fused causal-attention block (QKV proj + causal softmax attention + out proj
+ residual + LayerNorm) on 8 Trainium2 NeuronCores — bf16 v3 (pipelined).

Sharding: core c -> batch b = c//4, head-group r = c%4 (heads 4r..4r+3, local
model dims 256r..256r+256).  Each core computes Q/K/V for its head group over
its batch's full sequence and block-causal attention (no max subtraction --
scores are O(1)).  Output projection is row-parallel: per q-tile each core
computes the partial out-proj for ALL 2048 rows from its local normalized ctx;
a per-q-tile ReduceScatter over the batch's 4 cores sums the partials and
hands each core a 128-row shard, on which it does residual + LayerNorm.

v4 structure (vs v2, 336us -> ~281us on the profiled core):
- All input DMAs on ONE sync-queue ring in consumption-priority order
  (wq, x0, x1, wk, x2, x3, wv, tri, xres, wo): HBM bandwidth is shared, so
  concurrent loads starve the first matmul; serializing gets PE started ~13us
  (was ~25us).
- Host pre-swizzles every input into its SBUF layout so each DMA is a fully
  contiguous per-partition stream (strided rearranges cost ~3x in issue and
  transfer, and the inline-tensor tri mask cost a 10.6us engine-direct copy).
- QKV chunks interleave with attention q-tiles; q-tile order [0,1,3,2] so the
  longest (Scalar-exp-bound) qt3 block spreads over qt1/qt2's PE time and the
  final ReduceScatter is the cheaper qt2.  The dependency-driven Tile
  scheduler fills the exp-paced PE gaps with QKV/out-proj matmuls.
- PSUM: qk pool 2x1 bank (QKV groups / out-proj halves / bcast), scores 1x2
  banks (PE self-paces against the Scalar exp stream), AV accum 2x2 banks.
- Q/K PSUM evacuations on Scalar (idle in the QKV window; gpsimd cannot read
  PSUM), V evacuations + masks + normalize + y-casts on Vector, exp + LN
  rstd on Scalar.
- Causal masks on Vector: the gpsimd queue carries the RS triggers, which
  block on the previous collective's completion (straight-line ordering), so
  nothing latency-critical may sit behind them.
- Broadcast of 1/den over 64 partitions as a col-tiled concurrent matmul pair
  into one PSUM bank.
- LN tails carry tile_wait_until hints pushing them past all compute in every
  engine's static order (a mid-stream vector op gated on a ReduceScatter
  head-blocks masks -> exp -> PE for ~30us).  yr loads for the three already-
  finished RS tiles go out eagerly and back-to-back (gpsimd) so their LN
  chains execute inside the LAST collective's ~22us transfer window; only
  qt2's chain (loaded via the fast sync HWDGE path) runs after the wire.
- Per-qc y stores: the final RS trigger waits only on the last 256KB of
  staging instead of the full 1MB.
- Cross-core launch skew (~10-30us, run-dependent) leaks into every RS via
  peer-trigger waits; per-core compute-path cuts still help 1:1 since the
  laggard runs the same kernel.
"""

import numpy as np

B, N, D = 2, 2048, 1024
H, DH = 16, 64
NCORES = 8
HPC = 4          # heads per core
DP = HPC * DH    # 256 local model dims per core
NQ = N // 4      # 512 rows per q-tile
LN_EPS = 1e-5
GROUPS = [[0, 1, 2, 3], [4, 5, 6, 7]]

_CACHE = {}


def _build(flags):
    """Build+compile the Bacc program. flags = (has_qkv_bias, has_gamma, has_beta)."""
    import concourse.bass as bass
    import concourse.bacc as bacc
    import concourse.tile as tile
    from concourse import mybir
    from contextlib import ExitStack

    has_qkv_bias, has_gamma, has_beta = flags
    f32 = mybir.dt.float32
    bf16 = mybir.dt.bfloat16
    AF = mybir.ActivationFunctionType
    ALU = mybir.AluOpType

    nc = bacc.Bacc(
        trn_type="TRN2",
        target_bir_lowering=False,
        debug=False,
        num_devices=NCORES,
    )

    xTc = nc.dram_tensor("xTc", [4, 128, 8, 512], bf16, kind="ExternalInput").ap()
    xres = nc.dram_tensor("xres", [128, 4, D], bf16, kind="ExternalInput").ap()
    wqT = nc.dram_tensor("wqT", [128, 8, DP], bf16, kind="ExternalInput").ap()
    wkT = nc.dram_tensor("wkT", [128, 8, DP], bf16, kind="ExternalInput").ap()
    wvT = nc.dram_tensor("wvT", [128, 8, DP], bf16, kind="ExternalInput").ap()
    woL = nc.dram_tensor("woL", [128, 2, D], bf16, kind="ExternalInput").ap()
    tri_d = nc.dram_tensor("tri", [128, 128], bf16, kind="ExternalInput").ap()
    out = nc.dram_tensor("out", [4, 128, D], f32, kind="ExternalOutput").ap()
    if has_qkv_bias:
        bqkv = nc.dram_tensor("bqkv", [1, 3, DP], bf16, kind="ExternalInput").ap()
    if has_gamma:
        gamma_d = nc.dram_tensor("gamma", [D], f32, kind="ExternalInput").ap()
    if has_beta:
        beta_d = nc.dram_tensor("beta", [D], f32, kind="ExternalInput").ap()

    with tile.TileContext(nc) as tc, ExitStack() as ctx, \
            nc.allow_low_precision(reason="bf16 compute; gate is 2e-2"):
        singles = ctx.enter_context(tc.tile_pool(name="singles", bufs=1))

        wq_sb = singles.tile([128, 8, DP], bf16, tag="wq")
        wk_sb = singles.tile([128, 8, DP], bf16, tag="wk")
        wv_sb = singles.tile([128, 8, DP], bf16, tag="wv")
        wo_sb = singles.tile([128, 2, D], bf16, tag="wo")
        xres_sb = singles.tile([128, 4, D], bf16, tag="xres")
        tri_sb = singles.tile([128, 128], bf16, tag="tri")
        xT_sb = singles.tile([128, 4, 8, 512], bf16, tag="xT")

        # one serialized ring, consumption-priority order
        nc.sync.dma_start(wq_sb, wqT)
        nc.sync.dma_start(xT_sb[:, 0], xTc[0])
        nc.sync.dma_start(xT_sb[:, 1], xTc[1])
        nc.sync.dma_start(wk_sb, wkT)
        nc.sync.dma_start(xT_sb[:, 2], xTc[2])
        nc.sync.dma_start(xT_sb[:, 3], xTc[3])
        nc.sync.dma_start(wv_sb, wvT)
        nc.sync.dma_start(tri_sb, tri_d)
        nc.sync.dma_start(xres_sb, xres)
        nc.sync.dma_start(wo_sb, woL)

        ones_f32 = singles.tile([128, 64], f32, tag="ones_f32")
        nc.vector.memset(ones_f32, 1.0)
        ones64 = singles.tile([1, 64], bf16, tag="ones64")
        nc.vector.tensor_copy(out=ones64, in_=ones_f32[0:1, :])
        eps_sb = singles.tile([128, 1], f32, tag="eps")
        nc.vector.memset(eps_sb, LN_EPS)
        if has_qkv_bias:
            o512f = singles.tile([1, 512], f32, tag="o512f")
            nc.vector.memset(o512f, 1.0)
            ones512 = singles.tile([1, 512], bf16, tag="ones512")
            nc.vector.tensor_copy(out=ones512, in_=o512f)
            bqkv_sb = singles.tile([1, 3, DP], bf16, tag="bqkv")
            nc.scalar.dma_start(bqkv_sb, bqkv)
        if has_gamma:
            gamma_sb = singles.tile([128, D], f32, tag="gamma")
            nc.scalar.dma_start(
                gamma_sb,
                bass.AP(tensor=gamma_d.tensor, offset=gamma_d.offset,
                        ap=[[0, 128]] + gamma_d.ap),
            )
        if has_beta:
            beta_sb = singles.tile([128, D], f32, tag="beta")
            nc.scalar.dma_start(
                beta_sb,
                bass.AP(tensor=beta_d.tensor, offset=beta_d.offset,
                        ap=[[0, 128]] + beta_d.ap),
            )

        # persistent activations
        qT_sb = singles.tile([128, 2, N], bf16, tag="qT")   # Q^T [d'(256), n]
        kT_sb = singles.tile([128, 2, N], bf16, tag="kT")   # K^T [d'(256), n]
        # V + denominator-ones column at index 64 (partition bases must be
        # 32-aligned, so ctx must occupy PSUM partitions 0:64).  Width 66:
        # odd-width partial slices break HW ldweights addressing; col 65
        # stays 0 -> junk-free row.
        v_sb = singles.tile([128, 16, HPC, DH + 2], bf16, tag="v")
        ctx_sb = singles.tile([128, 2, N], bf16, tag="ctxT")  # normalized ctx^T
        nc.vector.memset(v_sb, 0.0)
        nc.vector.tensor_copy(
            out=v_sb[:, :, :, DH:DH + 1],
            in_=ones_f32.rearrange("p (a b c) -> p a b c", a=16, b=4))

        dram_pool = ctx.enter_context(tc.tile_pool(name="dram", bufs=1,
                                                   space="DRAM"))
        y_dram = [dram_pool.tile([NQ, D], bf16, tag=f"y{qt}", name=f"y{qt}")
                  for qt in range(4)]
        yr_dram = [dram_pool.tile([128, D], bf16, tag=f"yr{qt}",
                                  name=f"yr{qt}")
                   for qt in range(4)]

        # PSUM: 2x1 + 1x2 + 2x2 = 8 banks exactly
        qk_ps = ctx.enter_context(tc.tile_pool(name="qkps", bufs=2,
                                               space="PSUM"))
        sp_ps = ctx.enter_context(tc.tile_pool(name="spps", bufs=1,
                                               space="PSUM"))
        av_ps = ctx.enter_context(tc.tile_pool(name="avps", bufs=2,
                                               space="PSUM"))

        es_pool = ctx.enter_context(tc.tile_pool(name="es", bufs=5))
        nrm_pool = ctx.enter_context(tc.tile_pool(name="nrm", bufs=3))
        y_pool = ctx.enter_context(tc.tile_pool(name="ysb", bufs=2))
        ln_pool = ctx.enter_context(tc.tile_pool(name="ln", bufs=4))

        from concourse.dve_ops import (
            RECIP_APPROX_FAST_CONSTS,
            RECIPROCAL_APPROX_FAST,
        )

        def emit_qkv(ci):
            # Q, K: [256 d', 512 n] in two 128-row halves
            for wsb, dst, bidx in ((wq_sb, qT_sb, 0), (wk_sb, kT_sb, 1)):
                for dt_ in range(2):
                    ps = qk_ps.tile([128, 512], f32, tag="qk",
                                    name=f"qk_{bidx}_{ci}_{dt_}")
                    for ko in range(8):
                        nc.tensor.matmul(
                            ps,
                            lhsT=wsb[:, ko, 128 * dt_:128 * dt_ + 128],
                            rhs=xT_sb[:, ci, ko, :],
                            start=(ko == 0),
                            stop=(ko == 7 and not has_qkv_bias),
                        )
                    if has_qkv_bias:
                        nc.tensor.matmul(
                            ps,
                            lhsT=bqkv_sb[:, bidx, 128 * dt_:128 * dt_ + 128],
                            rhs=ones512,
                            start=False, stop=True,
                        )
                    # gpsimd can't read PSUM; Vector has more slack than
                    # Scalar in the QKV window (exp of qt0/qt1 already
                    # overlaps it under the interleaved schedule)
                    nc.vector.tensor_copy(
                        out=dst[:, dt_, 512 * ci:512 * (ci + 1)], in_=ps)
            # V: per 128-row subtile [128 n, 256 d']
            for s in range(4):
                nt = 4 * ci + s
                ps = qk_ps.tile([128, 512], f32, tag="qk",
                                name=f"v_{ci}_{s}")
                for ko in range(8):
                    nc.tensor.matmul(
                        ps[:, 0:DP],
                        lhsT=xT_sb[:, ci, ko, 128 * s:128 * s + 128],
                        rhs=wv_sb[:, ko],
                        start=(ko == 0),
                        stop=(ko == 7 and not has_qkv_bias),
                    )
                if has_qkv_bias:
                    nc.tensor.matmul(
                        ps[:, 0:DP],
                        lhsT=ones512[:, 0:128],
                        rhs=bqkv_sb[:, 2, :],
                        start=False, stop=True,
                    )
                nc.vector.tensor_copy(
                    out=v_sb[:, nt, :, 0:DH],
                    in_=ps[:, 0:DP].rearrange("p (h d) -> p h d", h=HPC))

        def emit_av(cps, heads, pend_item, n_kt):
            es, kt, c0 = pend_item
            for hi, h in enumerate(heads):
                nc.tensor.matmul(
                    cps[0:66, hi, c0:512],
                    lhsT=v_sb[:, kt, h, :],
                    rhs=es[:, hi, c0:512],
                    start=(kt == 0),
                    stop=(kt == n_kt - 1),
                )

        def emit_attn(qt):
            for hp in range(2):
                heads = (2 * hp, 2 * hp + 1)
                cps = av_ps.tile([128, 2, 512], f32, tag="ctx",
                                 name=f"cps_{qt}_{hp}")
                n_kt = 4 * qt + 4
                pend = []
                for kt in range(n_kt):
                    j = kt - 4 * qt
                    c0 = 128 * j if j > 0 else 0
                    sp = sp_ps.tile([128, 2, 512], f32, tag="s",
                                    name=f"sp_{qt}_{hp}_{kt}")
                    for hi, h in enumerate(heads):
                        ph = 64 * hi
                        nc.tensor.matmul(
                            sp[:, hi, c0:512],
                            lhsT=kT_sb[ph:ph + 64, hp,
                                       128 * kt:128 * kt + 128],
                            rhs=qT_sb[ph:ph + 64, hp,
                                      512 * qt + c0:512 * (qt + 1)],
                            start=True,
                            stop=True,
                        )
                    es = es_pool.tile([128, 2, 512], bf16, tag="es")
                    nc.scalar.activation(
                        out=es[:, :, c0:512],
                        in_=sp[:, :, c0:512],
                        func=AF.Exp, scale=0.125,
                    )
                    if j >= 0:
                        for hi in range(2):
                            nc.vector.tensor_mul(
                                out=es[:, hi, 128 * j:128 * j + 128],
                                in0=es[:, hi, 128 * j:128 * j + 128],
                                in1=tri_sb)
                    pend.append((es, kt, c0))
                    if len(pend) > 2:
                        emit_av(cps, heads, pend.pop(0), n_kt)
                while pend:
                    emit_av(cps, heads, pend.pop(0), n_kt)
                # denominator row to SBUF partition 0 (the custom-DVE op
                # ignores input base partitions on HW), then 1/den for both
                # heads in one fast custom-DVE op, cast to bf16, and
                # broadcast over 64 partitions as a col-tiled matmul pair
                den = nrm_pool.tile([1, 2, 512], f32, tag="den",
                                    name=f"den_{qt}_{hp}")
                nc.vector.tensor_copy(out=den, in_=cps[64:65, :, :])
                # custom-DVE writes bf16 directly: single-partition ops run
                # at 1 elem/cycle total, so dropping the separate bf16 cast
                # saves ~1.1us of chain latency per block
                rec = nrm_pool.tile([1, 2, 512], bf16, tag="rec",
                                    name=f"rec_{qt}_{hp}")
                cc = RECIP_APPROX_FAST_CONSTS
                nc.vector._custom_dve(
                    RECIPROCAL_APPROX_FAST, out=rec, in0=den,
                    s0=cc["s0"], s1=cc["s1"], imm2=cc["imm2"])
                bc = qk_ps.tile([128, 512], f32, tag="qk",
                                name=f"bc_{qt}_{hp}")
                nc.tensor.matmul(bc[0:64, :], lhsT=ones64,
                                 rhs=rec[0:1, 0, :], start=True, stop=True,
                                 tile_position=(0, 0))
                nc.tensor.matmul(bc[64:128, :], lhsT=ones64,
                                 rhs=rec[0:1, 1, :], start=True, stop=True,
                                 tile_position=(0, 64))
                bcs = nrm_pool.tile([128, 512], bf16, tag="bcs",
                                    name=f"bcs_{qt}_{hp}")
                nc.vector.tensor_copy(out=bcs, in_=bc)
                for hi in range(2):
                    nc.vector.tensor_mul(
                        out=ctx_sb[64 * hi:64 * hi + 64, hp,
                                   512 * qt:512 * (qt + 1)],
                        in0=cps[0:64, hi, :],
                        in1=bcs[64 * hi:64 * hi + 64, :])

        def emit_outproj(qt):
            y_sb = y_pool.tile([128, 4, D], bf16, tag="y", name=f"y_sb{qt}")
            for qc in range(4):
                for Dt in range(2):
                    ops = qk_ps.tile([128, 512], f32, tag="qk",
                                     name=f"op_{qt}_{qc}_{Dt}")
                    for hp in range(2):
                        nc.tensor.matmul(
                            ops,
                            lhsT=ctx_sb[:, hp,
                                        512 * qt + 128 * qc:
                                        512 * qt + 128 * qc + 128],
                            rhs=wo_sb[:, hp, 512 * Dt:512 * Dt + 512],
                            start=(hp == 0), stop=(hp == 1),
                        )
                    nc.vector.tensor_copy(
                        out=y_sb[:, qc, 512 * Dt:512 * (Dt + 1)], in_=ops)
                # per-qc store: the collective trigger then only waits for
                # the LAST 256KB instead of the full 1MB staging
                nc.sync.dma_start(
                    y_dram[qt][128 * qc:128 * (qc + 1), :], y_sb[:, qc, :])
            nc.gpsimd.collective_compute(
                "ReduceScatter", ALU.add,
                replica_groups=GROUPS,
                ins=[y_dram[qt][:, :]],
                outs=[yr_dram[qt][:, :]],
            )

        yr_tiles = {}

        def emit_ln_load(qt, eng):
            yr_sb = ln_pool.tile([128, D], bf16, tag="yr", name=f"yr_sb{qt}")
            eng.dma_start(yr_sb, yr_dram[qt])
            yr_tiles[qt] = yr_sb

        def emit_lntail(qt):
            yr_sb = yr_tiles[qt]
            yt = ln_pool.tile([128, D], f32, tag="yt", name=f"yt{qt}")
            nc.vector.tensor_add(out=yt, in0=yr_sb, in1=xres_sb[:, qt])
            st = ln_pool.tile([128, 2, 6], f32, tag="st", name=f"st{qt}")
            nc.vector.bn_stats(out=st[:, 0], in_=yt[:, 0:512])
            nc.vector.bn_stats(out=st[:, 1], in_=yt[:, 512:1024])
            mv = ln_pool.tile([128, 2], f32, tag="mv", name=f"mv{qt}")
            nc.vector.bn_aggr(out=mv, in_=st)
            # rstd = exp(-0.5*ln(var+eps)): stays on the natural_log_exp
            # activation table set (same set as the attention exp)
            lnt = ln_pool.tile([128, 1], f32, tag="lnt", name=f"lnt{qt}")
            rstd = ln_pool.tile([128, 1], f32, tag="rstd", name=f"rstd{qt}")
            nc.scalar.activation(out=lnt, in_=mv[:, 1:2], func=AF.Ln,
                                 bias=eps_sb, scale=1.0)
            nc.scalar.activation(out=rstd, in_=lnt, func=AF.Exp, scale=-0.5)
            nc.vector.tensor_scalar(
                out=yt, in0=yt, scalar1=mv[:, 0:1], scalar2=rstd,
                op0=ALU.subtract, op1=ALU.mult)
            if has_gamma:
                nc.vector.tensor_mul(out=yt, in0=yt, in1=gamma_sb)
            if has_beta:
                nc.vector.tensor_add(out=yt, in0=yt, in1=beta_sb)
            nc.gpsimd.dma_start(out[qt], yt)

        # q-tile order [0,1,3,2]: qt3 (the longest attention block, whose exp
        # stream is Scalar-bound) is emitted right after QKV completes so its
        # exp spreads across qt1/qt2's PE time, and the FINAL ReduceScatter
        # becomes the cheaper qt2.  Collective order is identical on every
        # core (same program), so straight-line ordering holds.
        emit_qkv(0)
        emit_qkv(1)
        qkv_left = [2, 3]
        for i, qt in enumerate([0, 1, 3, 2]):
            if qkv_left:
                emit_qkv(qkv_left.pop(0))
            emit_attn(qt)
            emit_outproj(qt)
        # LN tails at the very end of every engine's static order (wait-hint:
        # anything gated on a ReduceScatter must never sit ahead of attention
        # work in a queue -- head-of-line blocking).  yr loads for the three
        # already-completed RS tiles go out eagerly and back-to-back so their
        # LN chains all execute inside the LAST collective's ~22us transfer
        # window; only qt2's chain (loaded on the fast sync HWDGE path)
        # remains after the wire.
        with tc.tile_wait_until(1.0):
            for qt in (0, 1, 3):
                emit_ln_load(qt, nc.gpsimd)
            for qt in (0, 1, 3):
                emit_lntail(qt)
        with tc.tile_wait_until(1.2):
            emit_ln_load(2, nc.sync)
            emit_lntail(2)

    # Pin every activation to the natural_log_exp table set (covers exp, ln,
    # copy) so the Scalar engine never swaps tables between the attention exp
    # stream and the LayerNorm rstd (each swap costs ~1.5us + a pipe drain).
    import concourse.bacc as bacc_mod
    orig_tables = bacc_mod.get_activation_tables
    bacc_mod.get_activation_tables = lambda arch: {
        k: (v if k == "natural_log_exp_and_others" else set())
        for k, v in orig_tables(arch).items()
    }
    try:
        nc.compile()
    finally:
        bacc_mod.get_activation_tables = orig_tables
    return nc


def build_nc(flags=(False, False, False)):
    if flags not in _CACHE:
        _CACHE[flags] = _build(flags)
    return _CACHE[flags]


def make_in_maps(inputs):
    import ml_dtypes
    bf = ml_dtypes.bfloat16
    x = np.asarray(inputs["x"], dtype=np.float32)
    Wq = np.asarray(inputs["Wq"], np.float32)
    Wk = np.asarray(inputs["Wk"], np.float32)
    Wv = np.asarray(inputs["Wv"], np.float32)
    Wo = np.asarray(inputs["Wo"], np.float32)
    bq = np.asarray(inputs["bq"], np.float32)
    bk = np.asarray(inputs["bk"], np.float32)
    bv = np.asarray(inputs["bv"], np.float32)
    bo = np.asarray(inputs["bo"], np.float32)
    gamma = np.asarray(inputs["ln_gamma"], np.float32)
    beta = np.asarray(inputs["ln_beta"], np.float32)

    has_qkv_bias = bool(np.any(bq) or np.any(bk) or np.any(bv))
    has_gamma = not np.allclose(gamma, 1.0)
    has_beta = bool(np.any(beta))
    flags = (has_qkv_bias, has_gamma, has_beta)

    xres_full = x + bo  # residual with output bias folded in
    WoT = np.ascontiguousarray(Wo.T)  # [Dmodel, Dout]
    tri_np = np.ascontiguousarray(
        np.triu(np.ones((128, 128), np.float32)).astype(bf))

    def swizzle_w(Wslice):  # [1024, m] -> [128, 8, m] (p, ko, m)
        m = Wslice.shape[1]
        return np.ascontiguousarray(
            Wslice.reshape(8, 128, m).transpose(1, 0, 2).astype(bf))

    in_maps = []
    for c in range(NCORES):
        b, r = c // 4, c % 4
        cols = slice(DP * r, DP * (r + 1))
        xT = x[b].T  # [1024, 2048]
        xT_r = xT.reshape(8, 128, N).transpose(1, 0, 2)  # [128, 8, 2048]
        xTc = np.ascontiguousarray(
            np.stack([xT_r[:, :, 512 * ci:512 * (ci + 1)] for ci in range(4)]
                     ).astype(bf))  # [4, 128, 8, 512]
        # rows for this core: for each qt, rows 512*qt + 128*r .. +128
        xres_c = np.ascontiguousarray(
            xres_full[b].reshape(4, 4, 128, D)[:, r]
            .transpose(1, 0, 2).astype(bf))  # [128, 4, D]
        m = {
            "xTc": xTc,
            "xres": xres_c,
            "wqT": swizzle_w(Wq[cols, :].T),
            "wkT": swizzle_w(Wk[cols, :].T),
            "wvT": swizzle_w(Wv[cols, :].T),
            "woL": np.ascontiguousarray(
                WoT[cols, :].reshape(2, 128, D).transpose(1, 0, 2)
                .astype(bf)),
            "tri": tri_np,
        }
        if has_qkv_bias:
            m["bqkv"] = np.ascontiguousarray(
                np.stack([bq[cols], bk[cols], bv[cols]])[None].astype(bf))
        if has_gamma:
            m["gamma"] = gamma
        if has_beta:
            m["beta"] = beta
        in_maps.append(m)
    return flags, in_maps


def assemble(results):
    """results: list of per-core dicts with 'out' [4, 128, 1024]."""
    full = np.empty((B, N, D), dtype=np.float32)
    for c in range(NCORES):
        b, r = c // 4, c % 4
        o = results[c]["out"]
        for qt in range(4):
            full[b, NQ * qt + 128 * r:NQ * qt + 128 * (r + 1)] = o[qt]
    return full


def kernel(**inputs):
    from concourse.bass_utils import run_bass_kernel_spmd

    flags, in_maps = make_in_maps(inputs)
    nc = build_nc(flags)
    res = run_bass_kernel_spmd(nc, in_maps, core_ids=list(range(NCORES)))
    return assemble(res.results)


# revision 54
# speedup vs baseline: 1.1614x; 1.1614x over previous
"""Fused causal-attention block (QKV proj + causal softmax attention + out proj
+ residual + LayerNorm) on 8 Trainium2 NeuronCores — bf16 v3 (pipelined).

Sharding: core c -> batch b = c//4, head-group r = c%4 (heads 4r..4r+3, local
model dims 256r..256r+256).  Each core computes Q/K/V for its head group over
its batch's full sequence and block-causal attention (no max subtraction --
scores are O(1)).  Output projection is row-parallel: per q-tile each core
computes the partial out-proj for ALL 2048 rows from its local normalized ctx;
a per-q-tile ReduceScatter over the batch's 4 cores sums the partials and
hands each core a 128-row shard, on which it does residual + LayerNorm.

v4 structure (vs v2, 336us -> ~281us on the profiled core):
- All input DMAs on ONE sync-queue ring in consumption-priority order
  (wq, x0, x1, wk, x2, x3, wv, tri, xres, wo): HBM bandwidth is shared, so
  concurrent loads starve the first matmul; serializing gets PE started ~13us
  (was ~25us).
- Host pre-swizzles every input into its SBUF layout so each DMA is a fully
  contiguous per-partition stream (strided rearranges cost ~3x in issue and
  transfer, and the inline-tensor tri mask cost a 10.6us engine-direct copy).
- QKV chunks interleave with attention q-tiles; q-tile order [0,1,3,2] so the
  longest (Scalar-exp-bound) qt3 block spreads over qt1/qt2's PE time and the
  final ReduceScatter is the cheaper qt2.  The dependency-driven Tile
  scheduler fills the exp-paced PE gaps with QKV/out-proj matmuls.
- PSUM: qk pool 2x1 bank (QKV groups / out-proj halves / bcast), scores 1x2
  banks (PE self-paces against the Scalar exp stream), AV accum 2x2 banks.
- All PSUM evacuations + masks + normalize + y-casts on Vector (gpsimd
  cannot read PSUM; keeping Scalar exp-only paces attention best), exp + LN
  rstd on Scalar; the custom-DVE reciprocal writes bf16 directly.
- Causal masks on Vector: the gpsimd queue carries the RS triggers, which
  block on the previous collective's completion (straight-line ordering), so
  nothing latency-critical may sit behind them.
- Broadcast of 1/den over 64 partitions as a col-tiled concurrent matmul pair
  into one PSUM bank.
- LN tails carry tile_wait_until hints pushing them past all compute in every
  engine's static order (a mid-stream vector op gated on a ReduceScatter
  head-blocks masks -> exp -> PE for ~30us).  yr loads for the three already-
  finished RS tiles go out eagerly and back-to-back (gpsimd) so their LN
  chains execute inside the LAST collective's ~22us transfer window; only
  qt2's chain (loaded via the fast sync HWDGE path) runs after the wire.
- Per-qc y stores: the final RS trigger waits only on the last 256KB of
  staging instead of the full 1MB.
- Cross-core launch skew (~10-30us, run-dependent) leaks into every RS via
  peer-trigger waits; per-core compute-path cuts still help 1:1 since the
  laggard runs the same kernel.
"""

import numpy as np

B, N, D = 2, 2048, 1024
H, DH = 16, 64
NCORES = 8
HPC = 4          # heads per core
DP = HPC * DH    # 256 local model dims per core
NQ = N // 4      # 512 rows per q-tile
LN_EPS = 1e-5
GROUPS = [[0, 1, 2, 3], [4, 5, 6, 7]]

_CACHE = {}


def _build(flags):
    """Build+compile the Bacc program. flags = (has_qkv_bias, has_gamma, has_beta)."""
    import concourse.bass as bass
    import concourse.bacc as bacc
    import concourse.tile as tile
    from concourse import mybir
    from contextlib import ExitStack

    has_qkv_bias, has_gamma, has_beta = flags
    f32 = mybir.dt.float32
    bf16 = mybir.dt.bfloat16
    AF = mybir.ActivationFunctionType
    ALU = mybir.AluOpType

    nc = bacc.Bacc(
        trn_type="TRN2",
        target_bir_lowering=False,
        debug=False,
        num_devices=NCORES,
    )

    xTc = nc.dram_tensor("xTc", [4, 128, 8, 512], bf16, kind="ExternalInput").ap()
    xres = nc.dram_tensor("xres", [128, 4, D], bf16, kind="ExternalInput").ap()
    wqT = nc.dram_tensor("wqT", [128, 8, DP], bf16, kind="ExternalInput").ap()
    wkT = nc.dram_tensor("wkT", [128, 8, DP], bf16, kind="ExternalInput").ap()
    wvT = nc.dram_tensor("wvT", [128, 8, DP], bf16, kind="ExternalInput").ap()
    woL = nc.dram_tensor("woL", [128, 2, D], bf16, kind="ExternalInput").ap()
    tri_d = nc.dram_tensor("tri", [128, 128], bf16, kind="ExternalInput").ap()
    out = nc.dram_tensor("out", [4, 128, D], f32, kind="ExternalOutput").ap()
    if has_qkv_bias:
        bqkv = nc.dram_tensor("bqkv", [1, 3, DP], bf16, kind="ExternalInput").ap()
    if has_gamma:
        gamma_d = nc.dram_tensor("gamma", [D], f32, kind="ExternalInput").ap()
    if has_beta:
        beta_d = nc.dram_tensor("beta", [D], f32, kind="ExternalInput").ap()

    with tile.TileContext(nc) as tc, ExitStack() as ctx, \
            nc.allow_low_precision(reason="bf16 compute; gate is 2e-2"):
        singles = ctx.enter_context(tc.tile_pool(name="singles", bufs=1))

        wq_sb = singles.tile([128, 8, DP], bf16, tag="wq")
        wk_sb = singles.tile([128, 8, DP], bf16, tag="wk")
        wv_sb = singles.tile([128, 8, DP], bf16, tag="wv")
        wo_sb = singles.tile([128, 2, D], bf16, tag="wo")
        xres_sb = singles.tile([128, 4, D], bf16, tag="xres")
        tri_sb = singles.tile([128, 128], bf16, tag="tri")
        xT_sb = singles.tile([128, 4, 8, 512], bf16, tag="xT")

        # one serialized ring, consumption-priority order
        nc.sync.dma_start(wq_sb, wqT)
        nc.sync.dma_start(xT_sb[:, 0], xTc[0])
        nc.sync.dma_start(xT_sb[:, 1], xTc[1])
        nc.sync.dma_start(wk_sb, wkT)
        nc.sync.dma_start(xT_sb[:, 2], xTc[2])
        nc.sync.dma_start(xT_sb[:, 3], xTc[3])
        nc.sync.dma_start(wv_sb, wvT)
        nc.sync.dma_start(tri_sb, tri_d)
        nc.sync.dma_start(xres_sb, xres)
        nc.sync.dma_start(wo_sb, woL)

        ones_f32 = singles.tile([128, 64], f32, tag="ones_f32")
        nc.vector.memset(ones_f32, 1.0)
        ones64 = singles.tile([1, 64], bf16, tag="ones64")
        nc.vector.tensor_copy(out=ones64, in_=ones_f32[0:1, :])
        eps_sb = singles.tile([128, 1], f32, tag="eps")
        nc.vector.memset(eps_sb, LN_EPS)
        if has_qkv_bias:
            o512f = singles.tile([1, 512], f32, tag="o512f")
            nc.vector.memset(o512f, 1.0)
            ones512 = singles.tile([1, 512], bf16, tag="ones512")
            nc.vector.tensor_copy(out=ones512, in_=o512f)
            bqkv_sb = singles.tile([1, 3, DP], bf16, tag="bqkv")
            nc.scalar.dma_start(bqkv_sb, bqkv)
        if has_gamma:
            gamma_sb = singles.tile([128, D], f32, tag="gamma")
            nc.scalar.dma_start(
                gamma_sb,
                bass.AP(tensor=gamma_d.tensor, offset=gamma_d.offset,
                        ap=[[0, 128]] + gamma_d.ap),
            )
        if has_beta:
            beta_sb = singles.tile([128, D], f32, tag="beta")
            nc.scalar.dma_start(
                beta_sb,
                bass.AP(tensor=beta_d.tensor, offset=beta_d.offset,
                        ap=[[0, 128]] + beta_d.ap),
            )

        # persistent activations
        qT_sb = singles.tile([128, 2, N], bf16, tag="qT")   # Q^T [d'(256), n]
        kT_sb = singles.tile([128, 2, N], bf16, tag="kT")   # K^T [d'(256), n]
        # V + denominator-ones column at index 64 (partition bases must be
        # 32-aligned, so ctx must occupy PSUM partitions 0:64).  Width 66:
        # odd-width partial slices break HW ldweights addressing; col 65
        # stays 0 -> junk-free row.
        v_sb = singles.tile([128, 16, HPC, DH + 2], bf16, tag="v")
        ctx_sb = singles.tile([128, 2, N], bf16, tag="ctxT")  # normalized ctx^T
        nc.vector.memset(v_sb, 0.0)
        nc.vector.tensor_copy(
            out=v_sb[:, :, :, DH:DH + 1],
            in_=ones_f32.rearrange("p (a b c) -> p a b c", a=16, b=4))

        dram_pool = ctx.enter_context(tc.tile_pool(name="dram", bufs=1,
                                                   space="DRAM"))
        y_dram = [dram_pool.tile([NQ, D], bf16, tag=f"y{qt}", name=f"y{qt}")
                  for qt in range(4)]
        yr_dram = [dram_pool.tile([128, D], bf16, tag=f"yr{qt}",
                                  name=f"yr{qt}")
                   for qt in range(4)]

        # PSUM: 2x1 + 1x2 + 2x2 = 8 banks exactly
        qk_ps = ctx.enter_context(tc.tile_pool(name="qkps", bufs=2,
                                               space="PSUM"))
        sp_ps = ctx.enter_context(tc.tile_pool(name="spps", bufs=1,
                                               space="PSUM"))
        av_ps = ctx.enter_context(tc.tile_pool(name="avps", bufs=2,
                                               space="PSUM"))

        es_pool = ctx.enter_context(tc.tile_pool(name="es", bufs=5))
        nrm_pool = ctx.enter_context(tc.tile_pool(name="nrm", bufs=3))
        y_pool = ctx.enter_context(tc.tile_pool(name="ysb", bufs=2))
        ln_pool = ctx.enter_context(tc.tile_pool(name="ln", bufs=4))

        from concourse.dve_ops import (
            RECIP_APPROX_FAST_CONSTS,
            RECIPROCAL_APPROX_FAST,
        )

        def emit_qkv(ci):
            # Q, K: [256 d', 512 n] in two 128-row halves
            for wsb, dst, bidx in ((wq_sb, qT_sb, 0), (wk_sb, kT_sb, 1)):
                for dt_ in range(2):
                    ps = qk_ps.tile([128, 512], f32, tag="qk",
                                    name=f"qk_{bidx}_{ci}_{dt_}")
                    for ko in range(8):
                        nc.tensor.matmul(
                            ps,
                            lhsT=wsb[:, ko, 128 * dt_:128 * dt_ + 128],
                            rhs=xT_sb[:, ci, ko, :],
                            start=(ko == 0),
                            stop=(ko == 7 and not has_qkv_bias),
                        )
                    if has_qkv_bias:
                        nc.tensor.matmul(
                            ps,
                            lhsT=bqkv_sb[:, bidx, 128 * dt_:128 * dt_ + 128],
                            rhs=ones512,
                            start=False, stop=True,
                        )
                    # gpsimd can't read PSUM; Vector has more slack than
                    # Scalar in the QKV window (exp of qt0/qt1 already
                    # overlaps it under the interleaved schedule)
                    nc.vector.tensor_copy(
                        out=dst[:, dt_, 512 * ci:512 * (ci + 1)], in_=ps)
            # V: per 128-row subtile [128 n, 256 d']
            for s in range(4):
                nt = 4 * ci + s
                ps = qk_ps.tile([128, 512], f32, tag="qk",
                                name=f"v_{ci}_{s}")
                for ko in range(8):
                    nc.tensor.matmul(
                        ps[:, 0:DP],
                        lhsT=xT_sb[:, ci, ko, 128 * s:128 * s + 128],
                        rhs=wv_sb[:, ko],
                        start=(ko == 0),
                        stop=(ko == 7 and not has_qkv_bias),
                    )
                if has_qkv_bias:
                    nc.tensor.matmul(
                        ps[:, 0:DP],
                        lhsT=ones512[:, 0:128],
                        rhs=bqkv_sb[:, 2, :],
                        start=False, stop=True,
                    )
                nc.vector.tensor_copy(
                    out=v_sb[:, nt, :, 0:DH],
                    in_=ps[:, 0:DP].rearrange("p (h d) -> p h d", h=HPC))

        def emit_av(cps, heads, pend_item, n_kt):
            es, kt, c0 = pend_item
            for hi, h in enumerate(heads):
                nc.tensor.matmul(
                    cps[0:66, hi, c0:512],
                    lhsT=v_sb[:, kt, h, :],
                    rhs=es[:, hi, c0:512],
                    start=(kt == 0),
                    stop=(kt == n_kt - 1),
                )

        def emit_attn(qt):
            for hp in range(2):
                heads = (2 * hp, 2 * hp + 1)
                cps = av_ps.tile([128, 2, 512], f32, tag="ctx",
                                 name=f"cps_{qt}_{hp}")
                n_kt = 4 * qt + 4
                pend = []
                for kt in range(n_kt):
                    j = kt - 4 * qt
                    c0 = 128 * j if j > 0 else 0
                    sp = sp_ps.tile([128, 2, 512], f32, tag="s",
                                    name=f"sp_{qt}_{hp}_{kt}")
                    for hi, h in enumerate(heads):
                        ph = 64 * hi
                        nc.tensor.matmul(
                            sp[:, hi, c0:512],
                            lhsT=kT_sb[ph:ph + 64, hp,
                                       128 * kt:128 * kt + 128],
                            rhs=qT_sb[ph:ph + 64, hp,
                                      512 * qt + c0:512 * (qt + 1)],
                            start=True,
                            stop=True,
                        )
                    es = es_pool.tile([128, 2, 512], bf16, tag="es")
                    nc.scalar.activation(
                        out=es[:, :, c0:512],
                        in_=sp[:, :, c0:512],
                        func=AF.Exp, scale=0.125,
                    )
                    if j >= 0:
                        for hi in range(2):
                            nc.vector.tensor_mul(
                                out=es[:, hi, 128 * j:128 * j + 128],
                                in0=es[:, hi, 128 * j:128 * j + 128],
                                in1=tri_sb)
                    pend.append((es, kt, c0))
                    if len(pend) > 2:
                        emit_av(cps, heads, pend.pop(0), n_kt)
                while pend:
                    emit_av(cps, heads, pend.pop(0), n_kt)
                # denominator row to SBUF partition 0 (the custom-DVE op
                # ignores input base partitions on HW), then 1/den for both
                # heads in one fast custom-DVE op, cast to bf16, and
                # broadcast over 64 partitions as a col-tiled matmul pair
                den = nrm_pool.tile([1, 2, 512], f32, tag="den",
                                    name=f"den_{qt}_{hp}")
                nc.vector.tensor_copy(out=den, in_=cps[64:65, :, :])
                # custom-DVE writes bf16 directly: single-partition ops run
                # at 1 elem/cycle total, so dropping the separate bf16 cast
                # saves ~1.1us of chain latency per block
                rec = nrm_pool.tile([1, 2, 512], bf16, tag="rec",
                                    name=f"rec_{qt}_{hp}")
                cc = RECIP_APPROX_FAST_CONSTS
                nc.vector._custom_dve(
                    RECIPROCAL_APPROX_FAST, out=rec, in0=den,
                    s0=cc["s0"], s1=cc["s1"], imm2=cc["imm2"])
                bc = qk_ps.tile([128, 512], f32, tag="qk",
                                name=f"bc_{qt}_{hp}")
                nc.tensor.matmul(bc[0:64, :], lhsT=ones64,
                                 rhs=rec[0:1, 0, :], start=True, stop=True,
                                 tile_position=(0, 0))
                nc.tensor.matmul(bc[64:128, :], lhsT=ones64,
                                 rhs=rec[0:1, 1, :], start=True, stop=True,
                                 tile_position=(0, 64))
                bcs = nrm_pool.tile([128, 512], bf16, tag="bcs",
                                    name=f"bcs_{qt}_{hp}")
                nc.vector.tensor_copy(out=bcs, in_=bc)
                for hi in range(2):
                    nc.vector.tensor_mul(
                        out=ctx_sb[64 * hi:64 * hi + 64, hp,
                                   512 * qt:512 * (qt + 1)],
                        in0=cps[0:64, hi, :],
                        in1=bcs[64 * hi:64 * hi + 64, :])

        def emit_outproj(qt):
            y_sb = y_pool.tile([128, 4, D], bf16, tag="y", name=f"y_sb{qt}")
            for qc in range(4):
                for Dt in range(2):
                    ops = qk_ps.tile([128, 512], f32, tag="qk",
                                     name=f"op_{qt}_{qc}_{Dt}")
                    for hp in range(2):
                        nc.tensor.matmul(
                            ops,
                            lhsT=ctx_sb[:, hp,
                                        512 * qt + 128 * qc:
                                        512 * qt + 128 * qc + 128],
                            rhs=wo_sb[:, hp, 512 * Dt:512 * Dt + 512],
                            start=(hp == 0), stop=(hp == 1),
                        )
                    nc.vector.tensor_copy(
                        out=y_sb[:, qc, 512 * Dt:512 * (Dt + 1)], in_=ops)
                # per-qc store: the collective trigger then only waits for
                # the LAST 256KB instead of the full 1MB staging
                nc.sync.dma_start(
                    y_dram[qt][128 * qc:128 * (qc + 1), :], y_sb[:, qc, :])
            nc.gpsimd.collective_compute(
                "ReduceScatter", ALU.add,
                replica_groups=GROUPS,
                ins=[y_dram[qt][:, :]],
                outs=[yr_dram[qt][:, :]],
            )

        yr_tiles = {}

        def emit_ln_load(qt, eng):
            yr_sb = ln_pool.tile([128, D], bf16, tag="yr", name=f"yr_sb{qt}")
            eng.dma_start(yr_sb, yr_dram[qt])
            yr_tiles[qt] = yr_sb

        def emit_lntail(qt):
            yr_sb = yr_tiles[qt]
            yt = ln_pool.tile([128, D], f32, tag="yt", name=f"yt{qt}")
            nc.vector.tensor_add(out=yt, in0=yr_sb, in1=xres_sb[:, qt])
            st = ln_pool.tile([128, 2, 6], f32, tag="st", name=f"st{qt}")
            nc.vector.bn_stats(out=st[:, 0], in_=yt[:, 0:512])
            nc.vector.bn_stats(out=st[:, 1], in_=yt[:, 512:1024])
            mv = ln_pool.tile([128, 2], f32, tag="mv", name=f"mv{qt}")
            nc.vector.bn_aggr(out=mv, in_=st)
            # rstd = exp(-0.5*ln(var+eps)): stays on the natural_log_exp
            # activation table set (same set as the attention exp)
            lnt = ln_pool.tile([128, 1], f32, tag="lnt", name=f"lnt{qt}")
            rstd = ln_pool.tile([128, 1], f32, tag="rstd", name=f"rstd{qt}")
            nc.scalar.activation(out=lnt, in_=mv[:, 1:2], func=AF.Ln,
                                 bias=eps_sb, scale=1.0)
            nc.scalar.activation(out=rstd, in_=lnt, func=AF.Exp, scale=-0.5)
            nc.vector.tensor_scalar(
                out=yt, in0=yt, scalar1=mv[:, 0:1], scalar2=rstd,
                op0=ALU.subtract, op1=ALU.mult)
            if has_gamma:
                nc.vector.tensor_mul(out=yt, in0=yt, in1=gamma_sb)
            if has_beta:
                nc.vector.tensor_add(out=yt, in0=yt, in1=beta_sb)
            nc.gpsimd.dma_start(out[qt], yt)

        # q-tile order [0,1,3,2]: qt3 (the longest attention block, whose exp
        # stream is Scalar-bound) is emitted right after QKV completes so its
        # exp spreads across qt1/qt2's PE time, and the FINAL ReduceScatter
        # becomes the cheaper qt2.  Collective order is identical on every
        # core (same program), so straight-line ordering holds.
        emit_qkv(0)
        emit_qkv(1)
        qkv_left = [2, 3]
        for i, qt in enumerate([0, 1, 3, 2]):
            if qkv_left:
                emit_qkv(qkv_left.pop(0))
            emit_attn(qt)
            emit_outproj(qt)
        # LN tails at the very end of every engine's static order (wait-hint:
        # anything gated on a ReduceScatter must never sit ahead of attention
        # work in a queue -- head-of-line blocking).  yr loads for the three
        # already-completed RS tiles go out eagerly and back-to-back so their
        # LN chains all execute inside the LAST collective's ~22us transfer
        # window; only qt2's chain (loaded on the fast sync HWDGE path)
        # remains after the wire.
        with tc.tile_wait_until(1.0):
            for qt in (0, 1, 3):
                emit_ln_load(qt, nc.gpsimd)
            for qt in (0, 1, 3):
                emit_lntail(qt)
        with tc.tile_wait_until(1.2):
            emit_ln_load(2, nc.sync)
            emit_lntail(2)

    # Pin every activation to the natural_log_exp table set (covers exp, ln,
    # copy) so the Scalar engine never swaps tables between the attention exp
    # stream and the LayerNorm rstd (each swap costs ~1.5us + a pipe drain).
    import concourse.bacc as bacc_mod
    orig_tables = bacc_mod.get_activation_tables
    bacc_mod.get_activation_tables = lambda arch: {
        k: (v if k == "natural_log_exp_and_others" else set())
        for k, v in orig_tables(arch).items()
    }
    try:
        nc.compile()
    finally:
        bacc_mod.get_activation_tables = orig_tables
    return nc


def build_nc(flags=(False, False, False)):
    if flags not in _CACHE:
        _CACHE[flags] = _build(flags)
    return _CACHE[flags]


def make_in_maps(inputs):
    import ml_dtypes
    bf = ml_dtypes.bfloat16
    x = np.asarray(inputs["x"], dtype=np.float32)
    Wq = np.asarray(inputs["Wq"], np.float32)
    Wk = np.asarray(inputs["Wk"], np.float32)
    Wv = np.asarray(inputs["Wv"], np.float32)
    Wo = np.asarray(inputs["Wo"], np.float32)
    bq = np.asarray(inputs["bq"], np.float32)
    bk = np.asarray(inputs["bk"], np.float32)
    bv = np.asarray(inputs["bv"], np.float32)
    bo = np.asarray(inputs["bo"], np.float32)
    gamma = np.asarray(inputs["ln_gamma"], np.float32)
    beta = np.asarray(inputs["ln_beta"], np.float32)

    has_qkv_bias = bool(np.any(bq) or np.any(bk) or np.any(bv))
    has_gamma = not np.allclose(gamma, 1.0)
    has_beta = bool(np.any(beta))
    flags = (has_qkv_bias, has_gamma, has_beta)

    xres_full = x + bo  # residual with output bias folded in
    WoT = np.ascontiguousarray(Wo.T)  # [Dmodel, Dout]
    tri_np = np.ascontiguousarray(
        np.triu(np.ones((128, 128), np.float32)).astype(bf))

    def swizzle_w(Wslice):  # [1024, m] -> [128, 8, m] (p, ko, m)
        m = Wslice.shape[1]
        return np.ascontiguousarray(
            Wslice.reshape(8, 128, m).transpose(1, 0, 2).astype(bf))

    in_maps = []
    for c in range(NCORES):
        b, r = c // 4, c % 4
        cols = slice(DP * r, DP * (r + 1))
        xT = x[b].T  # [1024, 2048]
        xT_r = xT.reshape(8, 128, N).transpose(1, 0, 2)  # [128, 8, 2048]
        xTc = np.ascontiguousarray(
            np.stack([xT_r[:, :, 512 * ci:512 * (ci + 1)] for ci in range(4)]
                     ).astype(bf))  # [4, 128, 8, 512]
        # rows for this core: for each qt, rows 512*qt + 128*r .. +128
        xres_c = np.ascontiguousarray(
            xres_full[b].reshape(4, 4, 128, D)[:, r]
            .transpose(1, 0, 2).astype(bf))  # [128, 4, D]
        m = {
            "xTc": xTc,
            "xres": xres_c,
            "wqT": swizzle_w(Wq[cols, :].T),
            "wkT": swizzle_w(Wk[cols, :].T),
            "wvT": swizzle_w(Wv[cols, :].T),
            "woL": np.ascontiguousarray(
                WoT[cols, :].reshape(2, 128, D).transpose(1, 0, 2)
                .astype(bf)),
            "tri": tri_np,
        }
        if has_qkv_bias:
            m["bqkv"] = np.ascontiguousarray(
                np.stack([bq[cols], bk[cols], bv[cols]])[None].astype(bf))
        if has_gamma:
            m["gamma"] = gamma
        if has_beta:
            m["beta"] = beta
        in_maps.append(m)
    return flags, in_maps


def assemble(results):
    """results: list of per-core dicts with 'out' [4, 128, 1024]."""
    full = np.empty((B, N, D), dtype=np.float32)
    for c in range(NCORES):
        b, r = c // 4, c % 4
        o = results[c]["out"]
        for qt in range(4):
            full[b, NQ * qt + 128 * r:NQ * qt + 128 * (r + 1)] = o[qt]
    return full


def kernel(**inputs):
    from concourse.bass_utils import run_bass_kernel_spmd

    flags, in_maps = make_in_maps(inputs)
    nc = build_nc(flags)
    res = run_bass_kernel_spmd(nc, in_maps, core_ids=list(range(NCORES)))
    return assemble(res.results)


# revision 56
# speedup vs baseline: 1.1981x; 1.0316x over previous
"""Fused causal-attention block (QKV proj + causal softmax attention + out proj
+ residual + LayerNorm) on 8 Trainium2 NeuronCores — bf16 v3 (pipelined).

Sharding: core c -> batch b = c//4, head-group r = c%4 (heads 4r..4r+3, local
model dims 256r..256r+256).  Each core computes Q/K/V for its head group over
its batch's full sequence and block-causal attention (no max subtraction --
scores are O(1)).  Output projection is row-parallel: per q-tile each core
computes the partial out-proj for ALL 2048 rows from its local normalized ctx;
a per-q-tile ReduceScatter over the batch's 4 cores sums the partials and
hands each core a 128-row shard, on which it does residual + LayerNorm.

v4 structure (vs v2, 336us -> ~281us on the profiled core):
- All input DMAs on ONE sync-queue ring in consumption-priority order
  (wq, x0, x1, wk, x2, x3, wv, tri, xres, wo): HBM bandwidth is shared, so
  concurrent loads starve the first matmul; serializing gets PE started ~13us
  (was ~25us).
- Host pre-swizzles every input into its SBUF layout so each DMA is a fully
  contiguous per-partition stream (strided rearranges cost ~3x in issue and
  transfer, and the inline-tensor tri mask cost a 10.6us engine-direct copy).
- QKV chunks interleave with attention q-tiles; q-tile order [0,1,3,2] so the
  longest (Scalar-exp-bound) qt3 block spreads over qt1/qt2's PE time and the
  final ReduceScatter is the cheaper qt2.  The dependency-driven Tile
  scheduler fills the exp-paced PE gaps with QKV/out-proj matmuls.
- PSUM: qk pool 2x1 bank (QKV groups / out-proj halves / bcast), scores 1x2
  banks (PE self-paces against the Scalar exp stream), AV accum 2x2 banks.
- All PSUM evacuations + masks + normalize + y-casts on Vector (gpsimd
  cannot read PSUM; keeping Scalar exp-only paces attention best), exp + LN
  rstd on Scalar; the custom-DVE reciprocal writes bf16 directly.
- Causal masks on Vector: the gpsimd queue carries the RS triggers, which
  block on the previous collective's completion (straight-line ordering), so
  nothing latency-critical may sit behind them.
- Broadcast of 1/den over 64 partitions as a col-tiled concurrent matmul pair
  into one PSUM bank.
- LN tails carry tile_wait_until hints pushing them past all compute in every
  engine's static order (a mid-stream vector op gated on a ReduceScatter
  head-blocks masks -> exp -> PE for ~30us).  yr loads for the three already-
  finished RS tiles go out eagerly and back-to-back (gpsimd) so their LN
  chains execute inside the LAST collective's ~22us transfer window; only
  qt2's chain (loaded via the fast sync HWDGE path) runs after the wire.
- Per-qc y stores: the final RS trigger waits only on the last 256KB of
  staging instead of the full 1MB.
- Cross-core launch skew (~10-30us, run-dependent) leaks into every RS via
  peer-trigger waits; per-core compute-path cuts still help 1:1 since the
  laggard runs the same kernel.
"""

import numpy as np

B, N, D = 2, 2048, 1024
H, DH = 16, 64
NCORES = 8
HPC = 4          # heads per core
DP = HPC * DH    # 256 local model dims per core
NQ = N // 4      # 512 rows per q-tile
LN_EPS = 1e-5
GROUPS = [[0, 1, 2, 3], [4, 5, 6, 7]]

_CACHE = {}


def _build(flags):
    """Build+compile the Bacc program. flags = (has_qkv_bias, has_gamma, has_beta)."""
    import concourse.bass as bass
    import concourse.bacc as bacc
    import concourse.tile as tile
    from concourse import mybir
    from contextlib import ExitStack

    has_qkv_bias, has_gamma, has_beta = flags
    f32 = mybir.dt.float32
    bf16 = mybir.dt.bfloat16
    AF = mybir.ActivationFunctionType
    ALU = mybir.AluOpType

    nc = bacc.Bacc(
        trn_type="TRN2",
        target_bir_lowering=False,
        debug=False,
        num_devices=NCORES,
    )

    xTc = nc.dram_tensor("xTc", [4, 128, 8, 512], bf16, kind="ExternalInput").ap()
    xres = nc.dram_tensor("xres", [128, 4, D], bf16, kind="ExternalInput").ap()
    wqT = nc.dram_tensor("wqT", [128, 8, DP], bf16, kind="ExternalInput").ap()
    wkT = nc.dram_tensor("wkT", [128, 8, DP], bf16, kind="ExternalInput").ap()
    wvT = nc.dram_tensor("wvT", [128, 8, DP], bf16, kind="ExternalInput").ap()
    woL = nc.dram_tensor("woL", [128, 2, D], bf16, kind="ExternalInput").ap()
    tri_d = nc.dram_tensor("tri", [128, 128], bf16, kind="ExternalInput").ap()
    out = nc.dram_tensor("out", [4, 128, D], f32, kind="ExternalOutput").ap()
    if has_qkv_bias:
        bqkv = nc.dram_tensor("bqkv", [1, 3, DP], bf16, kind="ExternalInput").ap()
    if has_gamma:
        gamma_d = nc.dram_tensor("gamma", [D], f32, kind="ExternalInput").ap()
    if has_beta:
        beta_d = nc.dram_tensor("beta", [D], f32, kind="ExternalInput").ap()

    with tile.TileContext(nc) as tc, ExitStack() as ctx, \
            nc.allow_low_precision(reason="bf16 compute; gate is 2e-2"):
        singles = ctx.enter_context(tc.tile_pool(name="singles", bufs=1))

        wq_sb = singles.tile([128, 8, DP], bf16, tag="wq")
        wk_sb = singles.tile([128, 8, DP], bf16, tag="wk")
        wv_sb = singles.tile([128, 8, DP], bf16, tag="wv")
        wo_sb = singles.tile([128, 2, D], bf16, tag="wo")
        xres_sb = singles.tile([128, 4, D], bf16, tag="xres")
        tri_sb = singles.tile([128, 128], bf16, tag="tri")
        xT_sb = singles.tile([128, 4, 8, 512], bf16, tag="xT")

        # one serialized ring, consumption-priority order
        nc.sync.dma_start(wq_sb, wqT)
        nc.sync.dma_start(xT_sb[:, 0], xTc[0])
        nc.sync.dma_start(xT_sb[:, 1], xTc[1])
        nc.sync.dma_start(wk_sb, wkT)
        nc.sync.dma_start(xT_sb[:, 2], xTc[2])
        nc.sync.dma_start(xT_sb[:, 3], xTc[3])
        nc.sync.dma_start(wv_sb, wvT)
        nc.sync.dma_start(tri_sb, tri_d)
        nc.sync.dma_start(xres_sb, xres)
        nc.sync.dma_start(wo_sb, woL)

        ones_f32 = singles.tile([128, 64], f32, tag="ones_f32")
        nc.vector.memset(ones_f32, 1.0)
        ones64 = singles.tile([1, 64], bf16, tag="ones64")
        nc.vector.tensor_copy(out=ones64, in_=ones_f32[0:1, :])
        eps_sb = singles.tile([128, 1], f32, tag="eps")
        nc.vector.memset(eps_sb, LN_EPS)
        if has_qkv_bias:
            o512f = singles.tile([1, 512], f32, tag="o512f")
            nc.vector.memset(o512f, 1.0)
            ones512 = singles.tile([1, 512], bf16, tag="ones512")
            nc.vector.tensor_copy(out=ones512, in_=o512f)
            bqkv_sb = singles.tile([1, 3, DP], bf16, tag="bqkv")
            nc.scalar.dma_start(bqkv_sb, bqkv)
        if has_gamma:
            gamma_sb = singles.tile([128, D], f32, tag="gamma")
            nc.scalar.dma_start(
                gamma_sb,
                bass.AP(tensor=gamma_d.tensor, offset=gamma_d.offset,
                        ap=[[0, 128]] + gamma_d.ap),
            )
        if has_beta:
            beta_sb = singles.tile([128, D], f32, tag="beta")
            nc.scalar.dma_start(
                beta_sb,
                bass.AP(tensor=beta_d.tensor, offset=beta_d.offset,
                        ap=[[0, 128]] + beta_d.ap),
            )

        # persistent activations
        qT_sb = singles.tile([128, 2, N], bf16, tag="qT")   # Q^T [d'(256), n]
        kT_sb = singles.tile([128, 2, N], bf16, tag="kT")   # K^T [d'(256), n]
        # V + denominator-ones column at index 64 (partition bases must be
        # 32-aligned, so ctx must occupy PSUM partitions 0:64).  Width 66:
        # odd-width partial slices break HW ldweights addressing; col 65
        # stays 0 -> junk-free row.
        v_sb = singles.tile([128, 16, HPC, DH + 2], bf16, tag="v")
        ctx_sb = singles.tile([128, 2, N], bf16, tag="ctxT")  # normalized ctx^T
        nc.vector.memset(v_sb, 0.0)
        nc.vector.tensor_copy(
            out=v_sb[:, :, :, DH:DH + 1],
            in_=ones_f32.rearrange("p (a b c) -> p a b c", a=16, b=4))

        dram_pool = ctx.enter_context(tc.tile_pool(name="dram", bufs=1,
                                                   space="DRAM"))
        y_dram = [dram_pool.tile([NQ, D], bf16, tag=f"y{qt}", name=f"y{qt}")
                  for qt in range(4)]
        yr_dram = [dram_pool.tile([128, D], bf16, tag=f"yr{qt}",
                                  name=f"yr{qt}")
                   for qt in range(4)]

        # PSUM: 2x1 + 1x2 + 2x2 = 8 banks exactly
        qk_ps = ctx.enter_context(tc.tile_pool(name="qkps", bufs=2,
                                               space="PSUM"))
        sp_ps = ctx.enter_context(tc.tile_pool(name="spps", bufs=1,
                                               space="PSUM"))
        av_ps = ctx.enter_context(tc.tile_pool(name="avps", bufs=2,
                                               space="PSUM"))

        es_pool = ctx.enter_context(tc.tile_pool(name="es", bufs=5))
        nrm_pool = ctx.enter_context(tc.tile_pool(name="nrm", bufs=3))
        y_pool = ctx.enter_context(tc.tile_pool(name="ysb", bufs=2))
        ln_pool = ctx.enter_context(tc.tile_pool(name="ln", bufs=4))

        from concourse.dve_ops import (
            RECIP_APPROX_FAST_CONSTS,
            RECIPROCAL_APPROX_FAST,
        )

        def emit_qkv(ci):
            # Q, K: [256 d', 512 n] in two 128-row halves
            for wsb, dst, bidx in ((wq_sb, qT_sb, 0), (wk_sb, kT_sb, 1)):
                for dt_ in range(2):
                    ps = qk_ps.tile([128, 512], f32, tag="qk",
                                    name=f"qk_{bidx}_{ci}_{dt_}")
                    for ko in range(8):
                        nc.tensor.matmul(
                            ps,
                            lhsT=wsb[:, ko, 128 * dt_:128 * dt_ + 128],
                            rhs=xT_sb[:, ci, ko, :],
                            start=(ko == 0),
                            stop=(ko == 7 and not has_qkv_bias),
                        )
                    if has_qkv_bias:
                        nc.tensor.matmul(
                            ps,
                            lhsT=bqkv_sb[:, bidx, 128 * dt_:128 * dt_ + 128],
                            rhs=ones512,
                            start=False, stop=True,
                        )
                    # gpsimd can't read PSUM; Vector has more slack than
                    # Scalar in the QKV window (exp of qt0/qt1 already
                    # overlaps it under the interleaved schedule)
                    nc.vector.tensor_copy(
                        out=dst[:, dt_, 512 * ci:512 * (ci + 1)], in_=ps)
            # V: per 128-row subtile [128 n, 256 d']
            for s in range(4):
                nt = 4 * ci + s
                ps = qk_ps.tile([128, 512], f32, tag="qk",
                                name=f"v_{ci}_{s}")
                for ko in range(8):
                    nc.tensor.matmul(
                        ps[:, 0:DP],
                        lhsT=xT_sb[:, ci, ko, 128 * s:128 * s + 128],
                        rhs=wv_sb[:, ko],
                        start=(ko == 0),
                        stop=(ko == 7 and not has_qkv_bias),
                    )
                if has_qkv_bias:
                    nc.tensor.matmul(
                        ps[:, 0:DP],
                        lhsT=ones512[:, 0:128],
                        rhs=bqkv_sb[:, 2, :],
                        start=False, stop=True,
                    )
                nc.vector.tensor_copy(
                    out=v_sb[:, nt, :, 0:DH],
                    in_=ps[:, 0:DP].rearrange("p (h d) -> p h d", h=HPC))

        def emit_av(cps, heads, pend_item, n_kt):
            es, kt, c0 = pend_item
            for hi, h in enumerate(heads):
                nc.tensor.matmul(
                    cps[0:66, hi, c0:512],
                    lhsT=v_sb[:, kt, h, :],
                    rhs=es[:, hi, c0:512],
                    start=(kt == 0),
                    stop=(kt == n_kt - 1),
                )

        def emit_attn(qt):
            for hp in range(2):
                heads = (2 * hp, 2 * hp + 1)
                cps = av_ps.tile([128, 2, 512], f32, tag="ctx",
                                 name=f"cps_{qt}_{hp}")
                n_kt = 4 * qt + 4
                pend = []
                for kt in range(n_kt):
                    j = kt - 4 * qt
                    c0 = 128 * j if j > 0 else 0
                    sp = sp_ps.tile([128, 2, 512], f32, tag="s",
                                    name=f"sp_{qt}_{hp}_{kt}")
                    for hi, h in enumerate(heads):
                        ph = 64 * hi
                        nc.tensor.matmul(
                            sp[:, hi, c0:512],
                            lhsT=kT_sb[ph:ph + 64, hp,
                                       128 * kt:128 * kt + 128],
                            rhs=qT_sb[ph:ph + 64, hp,
                                      512 * qt + c0:512 * (qt + 1)],
                            start=True,
                            stop=True,
                        )
                    es = es_pool.tile([128, 2, 512], bf16, tag="es")
                    nc.scalar.activation(
                        out=es[:, :, c0:512],
                        in_=sp[:, :, c0:512],
                        func=AF.Exp, scale=0.125,
                    )
                    if j >= 0:
                        for hi in range(2):
                            nc.vector.tensor_mul(
                                out=es[:, hi, 128 * j:128 * j + 128],
                                in0=es[:, hi, 128 * j:128 * j + 128],
                                in1=tri_sb)
                    pend.append((es, kt, c0))
                    if len(pend) > 2:
                        emit_av(cps, heads, pend.pop(0), n_kt)
                while pend:
                    emit_av(cps, heads, pend.pop(0), n_kt)
                # denominator row to SBUF partition 0 (the custom-DVE op
                # ignores input base partitions on HW), then 1/den for both
                # heads in one fast custom-DVE op, cast to bf16, and
                # broadcast over 64 partitions as a col-tiled matmul pair
                # denominator row to SBUF partition 0 (the custom-DVE op
                # ignores input base partitions on HW -- verified: reading
                # cps[64:65] directly returns partition-0 garbage)
                den = nrm_pool.tile([1, 2, 512], f32, tag="den",
                                    name=f"den_{qt}_{hp}")
                nc.vector.tensor_copy(out=den, in_=cps[64:65, :, :])
                # custom-DVE writes bf16 directly: single-partition ops run
                # at 1 elem/cycle total, so dropping the separate bf16 cast
                # saves ~1.1us of chain latency per block
                rec = nrm_pool.tile([1, 2, 512], bf16, tag="rec",
                                    name=f"rec_{qt}_{hp}")
                cc = RECIP_APPROX_FAST_CONSTS
                nc.vector._custom_dve(
                    RECIPROCAL_APPROX_FAST, out=rec, in0=den,
                    s0=cc["s0"], s1=cc["s1"], imm2=cc["imm2"])
                bc = qk_ps.tile([128, 512], f32, tag="qk",
                                name=f"bc_{qt}_{hp}")
                nc.tensor.matmul(bc[0:64, :], lhsT=ones64,
                                 rhs=rec[0:1, 0, :], start=True, stop=True,
                                 tile_position=(0, 0))
                nc.tensor.matmul(bc[64:128, :], lhsT=ones64,
                                 rhs=rec[0:1, 1, :], start=True, stop=True,
                                 tile_position=(0, 64))
                bcs = nrm_pool.tile([128, 512], bf16, tag="bcs",
                                    name=f"bcs_{qt}_{hp}")
                nc.vector.tensor_copy(out=bcs, in_=bc)
                for hi in range(2):
                    nc.vector.tensor_mul(
                        out=ctx_sb[64 * hi:64 * hi + 64, hp,
                                   512 * qt:512 * (qt + 1)],
                        in0=cps[0:64, hi, :],
                        in1=bcs[64 * hi:64 * hi + 64, :])

        def emit_outproj(qt):
            y_sb = y_pool.tile([128, 4, D], bf16, tag="y", name=f"y_sb{qt}")
            for qc in range(4):
                for Dt in range(2):
                    ops = qk_ps.tile([128, 512], f32, tag="qk",
                                     name=f"op_{qt}_{qc}_{Dt}")
                    for hp in range(2):
                        nc.tensor.matmul(
                            ops,
                            lhsT=ctx_sb[:, hp,
                                        512 * qt + 128 * qc:
                                        512 * qt + 128 * qc + 128],
                            rhs=wo_sb[:, hp, 512 * Dt:512 * Dt + 512],
                            start=(hp == 0), stop=(hp == 1),
                        )
                    nc.vector.tensor_copy(
                        out=y_sb[:, qc, 512 * Dt:512 * (Dt + 1)], in_=ops)
                # per-qc store: the collective trigger then only waits for
                # the LAST 256KB instead of the full 1MB staging
                nc.sync.dma_start(
                    y_dram[qt][128 * qc:128 * (qc + 1), :], y_sb[:, qc, :])
            nc.gpsimd.collective_compute(
                "ReduceScatter", ALU.add,
                replica_groups=GROUPS,
                ins=[y_dram[qt][:, :]],
                outs=[yr_dram[qt][:, :]],
            )

        yr_tiles = {}

        def emit_ln_load(qt, eng):
            yr_sb = ln_pool.tile([128, D], bf16, tag="yr", name=f"yr_sb{qt}")
            eng.dma_start(yr_sb, yr_dram[qt])
            yr_tiles[qt] = yr_sb

        def emit_lntail(qt):
            yr_sb = yr_tiles[qt]
            yt = ln_pool.tile([128, D], f32, tag="yt", name=f"yt{qt}")
            nc.vector.tensor_add(out=yt, in0=yr_sb, in1=xres_sb[:, qt])
            st = ln_pool.tile([128, 2, 6], f32, tag="st", name=f"st{qt}")
            nc.vector.bn_stats(out=st[:, 0], in_=yt[:, 0:512])
            nc.vector.bn_stats(out=st[:, 1], in_=yt[:, 512:1024])
            mv = ln_pool.tile([128, 2], f32, tag="mv", name=f"mv{qt}")
            nc.vector.bn_aggr(out=mv, in_=st)
            # rstd = exp(-0.5*ln(var+eps)): stays on the natural_log_exp
            # activation table set (same set as the attention exp)
            lnt = ln_pool.tile([128, 1], f32, tag="lnt", name=f"lnt{qt}")
            rstd = ln_pool.tile([128, 1], f32, tag="rstd", name=f"rstd{qt}")
            nc.scalar.activation(out=lnt, in_=mv[:, 1:2], func=AF.Ln,
                                 bias=eps_sb, scale=1.0)
            nc.scalar.activation(out=rstd, in_=lnt, func=AF.Exp, scale=-0.5)
            nc.vector.tensor_scalar(
                out=yt, in0=yt, scalar1=mv[:, 0:1], scalar2=rstd,
                op0=ALU.subtract, op1=ALU.mult)
            if has_gamma:
                nc.vector.tensor_mul(out=yt, in0=yt, in1=gamma_sb)
            if has_beta:
                nc.vector.tensor_add(out=yt, in0=yt, in1=beta_sb)
            nc.gpsimd.dma_start(out[qt], yt)

        # q-tile order [0,1,3,2]: qt3 (the longest attention block, whose exp
        # stream is Scalar-bound) is emitted right after QKV completes so its
        # exp spreads across qt1/qt2's PE time, and the FINAL ReduceScatter
        # becomes the cheaper qt2.  Collective order is identical on every
        # core (same program), so straight-line ordering holds.
        emit_qkv(0)
        emit_qkv(1)
        qkv_left = [2, 3]
        for i, qt in enumerate([0, 1, 3, 2]):
            if qkv_left:
                emit_qkv(qkv_left.pop(0))
            emit_attn(qt)
            emit_outproj(qt)
        # LN tails at the very end of every engine's static order (wait-hint:
        # anything gated on a ReduceScatter must never sit ahead of attention
        # work in a queue -- head-of-line blocking).  yr loads for the three
        # already-completed RS tiles go out eagerly and back-to-back so their
        # LN chains all execute inside the LAST collective's ~22us transfer
        # window; only qt2's chain (loaded on the fast sync HWDGE path)
        # remains after the wire.
        with tc.tile_wait_until(1.0):
            for qt in (0, 1, 3):
                emit_ln_load(qt, nc.gpsimd)
            for qt in (0, 1, 3):
                emit_lntail(qt)
        with tc.tile_wait_until(1.2):
            emit_ln_load(2, nc.sync)
            emit_lntail(2)

    # Pin every activation to the natural_log_exp table set (covers exp, ln,
    # copy) so the Scalar engine never swaps tables between the attention exp
    # stream and the LayerNorm rstd (each swap costs ~1.5us + a pipe drain).
    import concourse.bacc as bacc_mod
    orig_tables = bacc_mod.get_activation_tables
    bacc_mod.get_activation_tables = lambda arch: {
        k: (v if k == "natural_log_exp_and_others" else set())
        for k, v in orig_tables(arch).items()
    }
    try:
        nc.compile()
    finally:
        bacc_mod.get_activation_tables = orig_tables
    return nc


def build_nc(flags=(False, False, False)):
    if flags not in _CACHE:
        _CACHE[flags] = _build(flags)
    return _CACHE[flags]


def make_in_maps(inputs):
    import ml_dtypes
    bf = ml_dtypes.bfloat16
    x = np.asarray(inputs["x"], dtype=np.float32)
    Wq = np.asarray(inputs["Wq"], np.float32)
    Wk = np.asarray(inputs["Wk"], np.float32)
    Wv = np.asarray(inputs["Wv"], np.float32)
    Wo = np.asarray(inputs["Wo"], np.float32)
    bq = np.asarray(inputs["bq"], np.float32)
    bk = np.asarray(inputs["bk"], np.float32)
    bv = np.asarray(inputs["bv"], np.float32)
    bo = np.asarray(inputs["bo"], np.float32)
    gamma = np.asarray(inputs["ln_gamma"], np.float32)
    beta = np.asarray(inputs["ln_beta"], np.float32)

    has_qkv_bias = bool(np.any(bq) or np.any(bk) or np.any(bv))
    has_gamma = not np.allclose(gamma, 1.0)
    has_beta = bool(np.any(beta))
    flags = (has_qkv_bias, has_gamma, has_beta)

    xres_full = x + bo  # residual with output bias folded in
    WoT = np.ascontiguousarray(Wo.T)  # [Dmodel, Dout]
    tri_np = np.ascontiguousarray(
        np.triu(np.ones((128, 128), np.float32)).astype(bf))

    def swizzle_w(Wslice):  # [1024, m] -> [128, 8, m] (p, ko, m)
        m = Wslice.shape[1]
        return np.ascontiguousarray(
            Wslice.reshape(8, 128, m).transpose(1, 0, 2).astype(bf))

    in_maps = []
    for c in range(NCORES):
        b, r = c // 4, c % 4
        cols = slice(DP * r, DP * (r + 1))
        xT = x[b].T  # [1024, 2048]
        xT_r = xT.reshape(8, 128, N).transpose(1, 0, 2)  # [128, 8, 2048]
        xTc = np.ascontiguousarray(
            np.stack([xT_r[:, :, 512 * ci:512 * (ci + 1)] for ci in range(4)]
                     ).astype(bf))  # [4, 128, 8, 512]
        # rows for this core: for each qt, rows 512*qt + 128*r .. +128
        xres_c = np.ascontiguousarray(
            xres_full[b].reshape(4, 4, 128, D)[:, r]
            .transpose(1, 0, 2).astype(bf))  # [128, 4, D]
        m = {
            "xTc": xTc,
            "xres": xres_c,
            "wqT": swizzle_w(Wq[cols, :].T),
            "wkT": swizzle_w(Wk[cols, :].T),
            "wvT": swizzle_w(Wv[cols, :].T),
            "woL": np.ascontiguousarray(
                WoT[cols, :].reshape(2, 128, D).transpose(1, 0, 2)
                .astype(bf)),
            "tri": tri_np,
        }
        if has_qkv_bias:
            m["bqkv"] = np.ascontiguousarray(
                np.stack([bq[cols], bk[cols], bv[cols]])[None].astype(bf))
        if has_gamma:
            m["gamma"] = gamma
        if has_beta:
            m["beta"] = beta
        in_maps.append(m)
    return flags, in_maps


def assemble(results):
    """results: list of per-core dicts with 'out' [4, 128, 1024]."""
    full = np.empty((B, N, D), dtype=np.float32)
    for c in range(NCORES):
        b, r = c // 4, c % 4
        o = results[c]["out"]
        for qt in range(4):
            full[b, NQ * qt + 128 * r:NQ * qt + 128 * (r + 1)] = o[qt]
    return full


def kernel(**inputs):
    from concourse.bass_utils import run_bass_kernel_spmd

    flags, in_maps = make_in_maps(inputs)
    nc = build_nc(flags)
    res = run_bass_kernel_spmd(nc, in_maps, core_ids=list(range(NCORES)))
    return assemble(res.results)
